# revision 63
# baseline (speedup 1.0000x reference)
"""BiRWKV7 TimeMix fused kernel for 8 TRN2 NeuronCores.

Sharding: core = dir*4 + b (dirs: 0=forward, 1=time-reversed). Each core runs
the full-T chunked DPLR WKV scan for one (batch, direction) stream; the
bidirectional combine swaps second-half ys between pair cores {b, b+4} via
four chunked bf16 AllReduces issued as the scan produces each block; host
reassembles halves. Projections/prep run in bf16 (PSUM f32), with phase-scoped
tile pools so peak SBUF stays under the 208 KiB/partition budget.
"""
import os
import numpy as np

B, T, C, H = 4, 1024, 512, 8
N = C // H
L = 64
G = T // L
NT = T // 128
NC = C // 128
TH = T // 2
LAM = float(np.exp(-0.5))
EPS_GN = 0.00064

# bf16 weight pack layout: (name, rows, cols)
_PACKB = [('wr', 128, 2048), ('wk', 128, 2048), ('wv', 128, 2048), ('wo', 128, 2048),
          ('w1t', 128, 512), ('a1t', 128, 512), ('g1t', 128, 1024), ('v1t', 128, 128),
          ('w2', 64, 512), ('a2', 64, 512), ('v2', 32, 512), ('g2', 128, 512),
          ('idb', 128, 128), ('rj', 128, 128), ('o2', 128, 2), ('e2', 2, 128),
          ('gw', 128, 32), ('mw', 64, 128), ('mm', 64, 64)]
_OFFB = {}
_c = 0
for _n, _r, _w in _PACKB:
    _OFFB[_n] = _c
    _c += _w
NB = _c
# f32 pack: vecs (4x12) | idt (128) | dirc (2)
NF = 48 + 128 + 2

_cache = {}


def _build():
    import contextlib
    import concourse.bass as bass
    import concourse.mybir as mybir
    import concourse.tile as tile
    from concourse import bacc

    f32 = mybir.dt.float32
    bf16 = mybir.dt.bfloat16
    AL = mybir.AluOpType
    AF = mybir.ActivationFunctionType

    nc = bacc.Bacc("TRN2", target_bir_lowering=False, debug=False,
                   enable_asserts=False, num_devices=8)

    def din(name, shape, dt=f32):
        return nc.dram_tensor(name, list(shape), dt, kind="ExternalInput").ap()

    xT = din("xT", (C, T), bf16); xxT = din("xxT", (C, T), bf16)
    vfT = din("vfT", (C, T), bf16)
    wpbd = din("wpbd", (128, NB), bf16)
    wpfd = din("wpfd", (128, NF))
    lng = din("lng", (1, C)); lnb = din("lnb", (1, C))
    out_d = nc.dram_tensor("out", [TH, C], f32, kind="ExternalOutput").ap()

    with tile.TileContext(nc) as tc:
        ctx = contextlib.ExitStack()
        with ctx:
            W = ctx.enter_context(tc.tile_pool(name="W", bufs=1))
            Q = ctx.enter_context(tc.tile_pool(name="Q", bufs=1, space="PSUM"))
            DR = ctx.enter_context(tc.tile_pool(name="dram", bufs=1, space="DRAM"))

            def wt(shape, dt, tag):
                return W.tile(list(shape), dt, tag=tag, name=tag)

            def q_(tag, shape, dt=f32):
                return Q.tile(list(shape), dt, tag=tag, name=tag)

            split_cc = os.environ.get('BIRWKV_SPLIT_CC', '1') == '1'
            if split_cc:
                cc_in = [DR.tile([128, C], bf16, name=f"cc_in{k}") for k in range(4)]
                cc_out = [DR.tile([128, C], bf16, name=f"cc_out{k}") for k in range(4)]
            else:
                cc_inF = DR.tile([TH, C], bf16, name="cc_in")
                cc_outF = DR.tile([TH, C], bf16, name="cc_out")
                cc_in = [cc_inF[:][128 * k:128 * (k + 1), :] for k in range(4)]
                cc_out = [cc_outF[:][128 * k:128 * (k + 1), :] for k in range(4)]

            # ---- persistent weights: two packed DMAs on separate queues ---
            wpb = wt([128, NB], bf16, "wpb")
            nc.scalar.dma_start(out=wpb[:], in_=wpbd)
            wpf = wt([128, NF], f32, "wpf")
            nc.gpsimd.dma_start(out=wpf[:], in_=wpfd)

            # helpers returning AP slices of the packs
            def WR(i): return wpb[:, _OFFB['wr'] + 512 * i:_OFFB['wr'] + 512 * (i + 1)]
            def WK(i): return wpb[:, _OFFB['wk'] + 512 * i:_OFFB['wk'] + 512 * (i + 1)]
            def WV(i): return wpb[:, _OFFB['wv'] + 512 * i:_OFFB['wv'] + 512 * (i + 1)]
            def WO(i): return wpb[:, _OFFB['wo'] + 512 * i:_OFFB['wo'] + 512 * (i + 1)]
            def W1(i): return wpb[:, _OFFB['w1t'] + 64 * i:_OFFB['w1t'] + 64 * (i + 1)]
            def A1(i): return wpb[:, _OFFB['a1t'] + 64 * i:_OFFB['a1t'] + 64 * (i + 1)]
            def G1(i): return wpb[:, _OFFB['g1t'] + 128 * i:_OFFB['g1t'] + 128 * (i + 1)]
            def V1(i): return wpb[:, _OFFB['v1t'] + 32 * i:_OFFB['v1t'] + 32 * (i + 1)]
            def W2(s): return wpb[0:64, _OFFB['w2'] + s.start:_OFFB['w2'] + s.stop]
            def A2(s): return wpb[0:64, _OFFB['a2'] + s.start:_OFFB['a2'] + s.stop]
            def V2(s): return wpb[0:32, _OFFB['v2'] + s.start:_OFFB['v2'] + s.stop]
            g2t = wpb[:, _OFFB['g2']:_OFFB['g2'] + 512]
            idb = wpb[:, _OFFB['idb']:_OFFB['idb'] + 128]
            rj = wpb[:, _OFFB['rj']:_OFFB['rj'] + 128]
            o2 = wpb[:, _OFFB['o2']:_OFFB['o2'] + 2]
            e2 = wpb[0:2, _OFFB['e2']:_OFFB['e2'] + 128]
            def GW(i): return wpb[:, _OFFB['gw'] + 8 * i:_OFFB['gw'] + 8 * (i + 1)]
            mW = wpb[0:64, _OFFB['mw']:_OFFB['mw'] + 128]
            mM = wpb[0:64, _OFFB['mm']:_OFFB['mm'] + 64]
            mWap = bass.AP(tensor=mW.tensor, offset=mW.offset,
                           ap=[list(mW.ap[0]), [0, 8], [64, 2], [1, 64]])
            mMap = bass.AP(tensor=mM.tensor, offset=mM.offset,
                           ap=[list(mM.ap[0]), [0, 8], [1, 64]])

            def vcol(i, j):
                return wpf[:, 12 * i + j:12 * i + j + 1]
            idt = wpf[:, 48:176]
            dc = wpf[:, 176:178]

            lngb = wt([128, C], f32, "lngb")
            lnbb = wt([128, C], f32, "lnbb")
            nc.gpsimd.dma_start(out=lngb[:], in_=bass.AP(tensor=lng.tensor, offset=lng.offset,
                                                         ap=[[0, 128]] + list(lng.ap[1:])))
            nc.gpsimd.dma_start(out=lnbb[:], in_=bass.AP(tensor=lnb.tensor, offset=lnb.offset,
                                                         ap=[[0, 128]] + list(lnb.ap[1:])))

            e24 = wt([128, 1], f32, "e24"); nc.gpsimd.memset(e24[:], 1e-24)
            egn = wt([128, 1], f32, "egn"); nc.gpsimd.memset(egn[:], EPS_GN)
            zero1 = wt([128, 1], f32, "zero1"); nc.gpsimd.memset(zero1[:], 0.0)

            # ---- persistent outputs of prep -----------------------------
            hg_ = wt([128, T], bf16, "hg")
            gate_t = wt([128, 64], f32, "gatet")
            arv = [wt([128, 2 * T], bf16, f"ar{i}") for i in range(NC)]
            ktl = [wt([128, T], bf16, f"ktl{i}") for i in range(NC)]
            btl = [wt([128, T], bf16, f"btl{i}") for i in range(NC)]
            ktt = [wt([64, C], bf16, f"ktt{g}") for g in range(G)]
            btt = [wt([64, C], bf16, f"btt{g}") for g in range(G)]
            vtt = [wt([64, C], bf16, f"vtt{g}") for g in range(G)]
            resid_t = [wt([128, C], f32, f"rst{t}") for t in range(4)]
            eLa = wt([128, NC * G], f32, "eLa")
            ys_t = [wt([128, C], f32 if t < 4 else bf16, f"ys{t}") for t in range(NT)]

            with tc.tile_pool(name="Pm", bufs=1) as Pm:
                def pm(shape, dt, tag):
                    return Pm.tile(list(shape), dt, tag=tag, name=tag)

                r_ = [pm([128, T], bf16, f"r{o}") for o in range(NC)]
                k_ = [pm([128, T], bf16, f"k{o}") for o in range(NC)]
                vf32 = [pm([128, T], bf16, f"vv{o}") for o in range(NC)]
                hw_ = pm([64, T], bf16, "hw")
                ha_ = pm([64, T], bf16, "ha")
                hv_ = pm([32, T], bf16, "hv")

                # =================== phase 1 =============================
                with tc.tile_pool(name="P1", bufs=1) as P1:
                    def p1(shape, dt, tag):
                        return P1.tile(list(shape), dt, tag=tag, name=tag)

                    xs = [p1([128, T], bf16, f"xs{i}") for i in range(NC)]
                    xxs = [p1([128, T], bf16, f"xxs{i}") for i in range(NC)]
                    for i in range(NC):
                        nc.sync.dma_start(out=xs[i][:], in_=xT[128 * i:128 * (i + 1), :])
                        nc.sync.dma_start(out=xxs[i][:], in_=xxT[128 * i:128 * (i + 1), :])

                    # gate from xx
                    pg = q_("q4", [8, T])
                    for fh in range(2):
                        fs = bass.ts(fh, 512)
                        for i in range(NC):
                            nc.tensor.matmul(pg[:, fs], GW(i), xxs[i][:, fs],
                                             start=(i == 0), stop=(i == NC - 1))
                    gts = p1([8, T], bf16, "gts")
                    nc.scalar.activation(gts[:], pg[:], AF.Sigmoid)
                    pgt = q_("q2", [128, 64], bf16)
                    for tt in range(NT):
                        nc.tensor.transpose(pgt[:, 8 * tt:8 * tt + 8],
                                            gts[:, 128 * tt:128 * (tt + 1)], idb[0:8, 0:8])
                    nc.scalar.activation(gate_t[:], pgt[:], AF.Copy)

                    # LoRA hiddens
                    def lora_in(wf, m, act, dst):
                        pt = q_("q4", [max(m, 2), T])
                        for fh in range(2):
                            fs = bass.ts(fh, 512)
                            for ci in range(8):
                                src = xs[ci][:, fs] if ci < 4 else xxs[ci - 4][:, fs]
                                nc.tensor.matmul(pt[:m, fs], wf(ci), src,
                                                 start=(ci == 0), stop=(ci == 7))
                        nc.scalar.activation(dst[:m, :], pt[:m, :], act)

                    lora_in(W1, 64, AF.Tanh, hw_)
                    lora_in(A1, 64, AF.Copy, ha_)
                    lora_in(G1, 128, AF.Sigmoid, hg_)
                    pv = q_("q4", [32, T])
                    for fh in range(2):
                        fs = bass.ts(fh, 512)
                        for ci in range(NC):
                            nc.tensor.matmul(pv[:, fs], V1(ci), xs[ci][:, fs],
                                             start=(ci == 0), stop=(ci == NC - 1))
                    nc.scalar.activation(hv_[:], pv[:], AF.Copy)

                    # mixes + projections
                    xm = [p1([128, T], bf16, f"xm{i}") for i in range(NC)]

                    def mix(col):
                        for i in range(NC):
                            nc.vector.scalar_tensor_tensor(
                                out=xm[i][:], in0=xxs[i][:], scalar=vcol(i, col),
                                in1=xs[i][:], op0=AL.mult, op1=AL.add)

                    def proj(wf, dsts):
                        for o in range(NC):
                            pt = q_("q0" if o % 2 == 0 else "q1", [128, T])
                            for fh in range(2):
                                fs = bass.ts(fh, 512)
                                for ci in range(NC):
                                    nc.tensor.matmul(pt[:, fs],
                                                     wf(ci)[:, 128 * o:128 * (o + 1)],
                                                     xm[ci][:, fs],
                                                     start=(ci == 0), stop=(ci == NC - 1))
                            nc.scalar.activation(dsts[o][:], pt[:], AF.Copy)

                    mix(0); proj(WR, r_)
                    mix(1); proj(WK, k_)
                    mix(2)
                    for o in range(NC):
                        pt = q_("q0" if o % 2 == 0 else "q1", [128, T])
                        for fh in range(2):
                            fs = bass.ts(fh, 512)
                            for ci in range(NC):
                                nc.tensor.matmul(pt[:, fs], WV(ci)[:, 128 * o:128 * (o + 1)],
                                                 xm[ci][:, fs],
                                                 start=(ci == 0), stop=(ci == NC - 1))
                        pgv = q_("q4", [128, T])
                        for fh in range(2):
                            fs = bass.ts(fh, 512)
                            nc.tensor.matmul(pgv[:, fs], V2(slice(128 * o, 128 * (o + 1))),
                                             hv_[:32, fs], start=True, stop=True)
                        gvv = p1([128, T], bf16, "gvv")
                        nc.scalar.activation(gvv[:], pgv[:], AF.Sigmoid, bias=vcol(o, 8))
                        vfo = p1([128, T], bf16, "vfs")
                        nc.sync.dma_start(out=vfo[:], in_=vfT[128 * o:128 * (o + 1), :])
                        vd = p1([128, T], bf16, "vd")
                        nc.vector.tensor_tensor(out=vd[:], in0=vfo[:], in1=pt[:], op=AL.subtract)
                        vm = p1([128, T], bf16, "vm")
                        nc.vector.tensor_tensor(out=vm[:], in0=vd[:], in1=gvv[:], op=AL.mult)
                        nc.vector.tensor_tensor(out=vf32[o][:], in0=pt[:], in1=vm[:], op=AL.add)

                # =================== phase 2: per-c-tile chain ===========
                with tc.tile_pool(name="P2", bufs=1) as P2:
                    def p2(shape, dt, tag):
                        return P2.tile(list(shape), dt, tag=tag, name=tag)

                    zb = bass.AP(tensor=zero1[:].tensor, offset=zero1[:].offset,
                                 ap=[list(zero1[:].ap[0]), [0, T]])
                    for o in range(NC):
                        sx = o % 2
                        tA = p2([128, T], f32, f"tA{sx}"); tB = p2([128, T], f32, f"tB{sx}")
                        tC = p2([128, T], f32, f"tC{sx}"); tD = p2([128, T], bf16, f"tD{sx}")
                        tE = p2([128, T], bf16, f"tE{sx}"); tF = p2([128, T], bf16, f"tF{sx}")
                        tG = p2([128, T], bf16, f"tG{sx}"); tH = p2([2, T], bf16, f"tH{sx}")
                        bs = p2([128, G], f32, f"bs{sx}")
                        v3 = lambda t: t[:].rearrange("p (g d) -> p g d", d=L)
                        ar3 = arv[o][:].rearrange("p (g d) -> p g d", d=2 * L)
                        # decay chain
                        ps = q_("q0", [128, T])
                        for fh in range(2):
                            fs = bass.ts(fh, 512)
                            nc.tensor.matmul(ps[:, fs], W2(slice(128 * o, 128 * (o + 1))),
                                             hw_[:64, fs], start=True, stop=True)
                        nc.scalar.activation(tA[:], ps[:], AF.Sigmoid, bias=vcol(o, 6))
                        nc.vector.tensor_tensor_scan(out=tB[:], data0=tA[:], data1=zb,
                                                     initial=0.0, op0=AL.add, op1=AL.add)
                        nc.gpsimd.memset(bs[:, 0:1], 0.0)
                        nc.vector.tensor_copy(out=bs[:, 1:G], in_=v3(tB)[:, 0:G - 1, L - 1])
                        bsb = bass.AP(tensor=bs[:].tensor, offset=bs[:].offset,
                                      ap=[list(bs[:].ap[0]), list(bs[:].ap[1]), [0, L]])
                        nc.vector.tensor_tensor(out=v3(tA), in0=v3(tB), in1=bsb, op=AL.subtract)
                        nc.scalar.activation(tC[:], tA[:], AF.Exp, scale=-LAM)
                        nc.scalar.activation(tD[:], tA[:], AF.Exp, scale=LAM)
                        nc.gpsimd.memset(v3(tE)[:, :, 0:1], 1.0)
                        nc.gpsimd.tensor_copy(out=v3(tE)[:, :, 1:L], in_=v3(tC)[:, :, 0:L - 1])
                        nc.vector.tensor_copy(out=eLa[:, o * G:(o + 1) * G],
                                              in_=v3(tC)[:, :, L - 1])
                        r3 = r_[o][:].rearrange("p (g d) -> p g d", d=L)
                        nc.vector.tensor_tensor(out=ar3[:, :, L:2 * L], in0=r3,
                                                in1=v3(tC), op=AL.mult)
                        # aicl -> ks, ktl
                        ps = q_("q1", [128, T])
                        for fh in range(2):
                            fs = bass.ts(fh, 512)
                            nc.tensor.matmul(ps[:, fs], A2(slice(128 * o, 128 * (o + 1))),
                                             ha_[:64, fs], start=True, stop=True)
                        nc.scalar.activation(tA[:], ps[:], AF.Sigmoid, bias=vcol(o, 7))
                        nc.vector.tensor_scalar(out=tB[:], in0=tA[:], scalar1=vcol(o, 4),
                                                scalar2=vcol(o, 5), op0=AL.mult, op1=AL.add)
                        nc.gpsimd.tensor_tensor(out=tF[:], in0=k_[o][:], in1=tB[:], op=AL.mult)
                        nc.gpsimd.tensor_tensor(out=ktl[o][:], in0=tF[:], in1=tD[:], op=AL.mult)
                        # kk norm -> kkn
                        nc.vector.tensor_scalar(out=tB[:], in0=k_[o][:], scalar1=vcol(o, 3),
                                                scalar2=None, op0=AL.mult)
                        nc.scalar.activation(tG[:], tB[:], AF.Square)
                        ps = q_("q4", [2, T])
                        for fh in range(2):
                            fs = bass.ts(fh, 512)
                            nc.tensor.matmul(ps[:, fs], o2,
                                             tG[:, fs], start=True, stop=True)
                        nc.scalar.activation(tH[:], ps[:], AF.Sqrt, bias=e24[0:2, :])
                        with nc.allow_low_precision(reason="bf16 inv-norm broadcast"):
                            nc.vector.reciprocal(out=tH[:], in_=tH[:])
                        pb = q_("q0", [128, T])
                        for fh in range(2):
                            fs = bass.ts(fh, 512)
                            nc.tensor.matmul(pb[:, fs], e2,
                                             tH[:, fs], start=True, stop=True)
                        nc.vector.tensor_tensor(out=tG[:], in0=tB[:], in1=pb[:], op=AL.mult)
                        nc.vector.scalar_tensor_tensor(out=ar3[:, :, 0:L], in0=v3(tG),
                                                       scalar=-1.0, in1=v3(tE),
                                                       op0=AL.mult, op1=AL.mult)
                        nc.vector.tensor_tensor(out=tB[:], in0=tG[:], in1=tA[:], op=AL.mult)
                        nc.gpsimd.tensor_tensor(out=btl[o][:], in0=tB[:], in1=tD[:], op=AL.mult)
                        # resid
                        nc.gpsimd.tensor_tensor(out=tC[:], in0=r_[o][:], in1=tF[:], op=AL.mult)
                        nc.vector.tensor_scalar(out=tG[:], in0=tC[:], scalar1=vcol(o, 9),
                                                scalar2=None, op0=AL.mult)
                        ps = q_("q4", [2, T])
                        for fh in range(2):
                            fs = bass.ts(fh, 512)
                            nc.tensor.matmul(ps[:, fs], o2,
                                             tG[:, fs], start=True, stop=True)
                        nc.scalar.activation(tH[:], ps[:], AF.Copy)
                        pb = q_("q0", [128, T])
                        for fh in range(2):
                            fs = bass.ts(fh, 512)
                            nc.tensor.matmul(pb[:, fs], e2,
                                             tH[:, fs], start=True, stop=True)
                        nc.vector.tensor_tensor(out=tC[:], in0=vf32[o][:], in1=pb[:], op=AL.mult)
                        # transposes to t-major
                        co = slice(128 * o, 128 * (o + 1))
                        for tt in range(NT):
                            cs = bass.ts(tt, 128)
                            for (srcs, dsts, qq) in ((ktl[o], ktt, "q2"), (btl[o], btt, "q3"),
                                                     (vf32[o], vtt, "q2")):
                                pa = q_(qq, [128, 128], bf16)
                                nc.tensor.transpose(pa[:], srcs[:, cs], idb)
                                nc.vector.tensor_copy(out=dsts[2 * tt][:, co], in_=pa[0:64, :])
                                nc.scalar.activation(dsts[2 * tt + 1][:, co], pa[64:128, :], AF.Copy)
                            if tt < 4:
                                pd = q_("q3", [128, 128])
                                nc.tensor.matmul(pd[:], tC[:, cs], idt,
                                                 is_transpose=True, start=True, stop=True)
                                nc.scalar.activation(resid_t[tt][:, co], pd[:], AF.Copy)
            # =================== WKV chunk loop ======================
            with tc.tile_pool(name="P3", bufs=1) as P3:
                def p3(shape, dt, tag):
                    return P3.tile(list(shape), dt, tag=tag, name=tag)

                s0t = p3([128, 256], f32, "s0t")
                nc.gpsimd.memset(s0t[:], 0.0)

                tp64 = os.environ.get('BIRWKV_TP64', '0') == '1'
                if not tp64:
                    # base-0 copies of odd-head halves (PE operand reads at
                    # partition base 64 fail on this toolchain)
                    kto = [p3([64, T], bf16, f"kto{i}") for i in range(NC)]
                    bto = [p3([64, T], bf16, f"bto{i}") for i in range(NC)]
                    aro = [p3([64, 2 * T], bf16, f"aro{i}") for i in range(NC)]
                    for i in range(NC):
                        nc.sync.dma_start(out=kto[i][:], in_=ktl[i][64:128, :])
                        nc.sync.dma_start(out=bto[i][:], in_=btl[i][64:128, :])
                        nc.sync.dma_start(out=aro[i][:], in_=arv[i][64:128, :])
                    aro3 = [aro[i][:].rearrange("p (g d) -> p g d", d=2 * L)
                            for i in range(NC)]

                def hsl(tens, h, g, w=L):
                    r0 = 64 * (h % 2)
                    if tp64 or h % 2 == 0:
                        return tens[h // 2][r0:r0 + 64, w * g:w * (g + 1)]
                    odd = {id(ktl): kto, id(btl): bto}[id(tens)]
                    return odd[h // 2][:, w * g:w * (g + 1)]

                def tmj(tens, g, h):  # t-major slice (64, 64) chunk g head h
                    return tens[g][:, 64 * h:64 * h + 64]

                ar3l = [arv[i][:].rearrange("p (g d) -> p g d", d=2 * L) for i in range(NC)]

                def arslice(h, g, lo=0, hi=2 * L):
                    if tp64 or h % 2 == 0:
                        r0 = 64 * (h % 2)
                        return ar3l[h // 2][r0:r0 + 64, g, lo:hi]
                    return aro3[h // 2][:, g, lo:hi]

                def tp(h):
                    return (64 * (h % 2), 0) if tp64 else None

                def prework(g):
                    pt1 = q_("q0", [64, 1024])
                    pt2 = q_("q1", [64, 1024])
                    ptm = q_("q2", [64, 512])
                    for h in range(8):
                        arsl = arslice(h, g)
                        nc.tensor.matmul(pt1[:, 128 * h:128 * h + 128], hsl(ktl, h, g),
                                         arsl, start=True, stop=True, tile_position=tp(h))
                        nc.tensor.matmul(pt2[:, 128 * h:128 * h + 128], hsl(btl, h, g),
                                         arsl, start=True, stop=True, tile_position=tp(h))
                        nc.tensor.matmul(ptm[:, 64 * h:64 * h + 64], arslice(h, g, 0, L),
                                         hsl(btl, h, g), start=True, stop=True,
                                         tile_position=tp(h))
                    return pt1, pt2, ptm

                def masks(g, pts):
                    pt1, pt2, ptm = pts
                    sxp = g % 2
                    wt1 = p3([64, 1024], bf16, f"wt1{sxp}")
                    wt2 = p3([64, 1024], bf16, f"wt2{sxp}")
                    wm = p3([64, 512], bf16, f"wm{sxp}")
                    r4 = lambda t: t.rearrange("p (h u d) -> p h u d", u=2, d=L)
                    nc.vector.tensor_tensor(out=r4(wt1[:]), in0=r4(pt1[:]), in1=mWap,
                                            op=AL.mult)
                    nc.vector.tensor_tensor(out=r4(wt2[:]), in0=r4(pt2[:]), in1=mWap,
                                            op=AL.mult)
                    nc.vector.tensor_tensor(out=wm[:].rearrange("p (h d) -> p h d", d=L),
                                            in0=ptm[:].rearrange("p (h d) -> p h d", d=L),
                                            in1=mMap, op=AL.mult)
                    return wt1, wt2, wm

                # software pipeline: prework/masks of chunk g+1/g+2 are emitted
                # inside iteration g so the PE always has state-independent
                # matmuls queued while the state chain runs on vector/scalar
                pw = {0: prework(0)}
                wts = {0: masks(0, pw.pop(0))}
                pw[1] = prework(1)
                for g in range(G):
                    tt, hf = g // 2, 64 * (g % 2)
                    wt1, wt2, wm = wts.pop(g)
                    if tp64:
                        s0b = p3([128, 256], bf16, "s0b")
                        nc.scalar.activation(s0b[:], s0t[:], AF.Copy)

                        def s0sl(h):
                            return s0b[64 * (h % 2):64 * (h % 2) + 64,
                                       64 * (h // 2):64 * (h // 2) + 64]
                    else:
                        s0e = p3([64, 256], bf16, "s0e")
                        s0o = p3([64, 256], bf16, "s0o")
                        nc.scalar.activation(s0e[:], s0t[0:64, :], AF.Copy)
                        nc.scalar.activation(s0o[:], s0t[64:128, :], AF.Copy)

                        def s0sl(h):
                            i = h // 2
                            return (s0e if h % 2 == 0 else s0o)[:, 64 * i:64 * i + 64]
                    pt3 = q_("q3", [128, 512])
                    psq_ = None
                    for h in range(8):
                        # start only on h==0: start=True clears has_written for the
                        # WHOLE bank, which would break accumulation into sibling
                        # head blocks written earlier in this group
                        nc.tensor.matmul(pt3[:, 64 * h:64 * h + 64], arslice(h, g),
                                         s0sl(h), start=(h == 0), stop=False,
                                         tile_position=tp(h))
                    for h in range(8):
                        nc.tensor.matmul(pt3[0:64, 64 * h:64 * h + 64],
                                         wt1[:, 128 * h:128 * h + 64],
                                         tmj(vtt, g, h), start=False, stop=False)
                    curM = [wm[:, 64 * h:64 * h + 64] for h in range(8)]
                    curMT = [wt2[:, 128 * h:128 * h + 64] for h in range(8)]
                    for lev in range(6):
                        xsb = p3([64, 512], bf16, f"xsb{lev % 2}")
                        if lev % 2 == 0:
                            nc.scalar.activation(xsb[:], pt3[0:64, :], AF.Copy)
                        else:
                            nc.vector.tensor_copy(out=xsb[:], in_=pt3[0:64, :])
                        for h in range(8):
                            nc.tensor.matmul(pt3[0:64, 64 * h:64 * h + 64], curMT[h],
                                             xsb[:, 64 * h:64 * h + 64], start=False, stop=False)
                        if lev < 5:
                            psq = q_("q4", [64, 1024])
                            for h in range(8):
                                nc.tensor.matmul(psq[:, 128 * h:128 * h + 64], curMT[h], curM[h],
                                                 start=True, stop=True)
                                nc.tensor.matmul(psq[:, 128 * h + 64:128 * h + 128], curM[h],
                                                 curMT[h], start=True, stop=True)
                            wq = p3([64, 1024], bf16, f"wq{lev % 2}")
                            if lev % 2 == 0:
                                nc.vector.tensor_copy(out=wq[:], in_=psq[:])
                            else:
                                nc.scalar.activation(wq[:], psq[:], AF.Copy)
                            curM = [wq[:, 128 * h:128 * h + 64] for h in range(8)]
                            curMT = [wq[:, 128 * h + 64:128 * h + 128] for h in range(8)]
                    us = p3([64, 512], bf16, "us")
                    nc.vector.tensor_copy(out=us[:], in_=pt3[0:64, :])
                    for h in range(8):
                        nc.tensor.matmul(pt3[64:128, 64 * h:64 * h + 64],
                                         wt2[:, 128 * h + 64:128 * h + 128],
                                         us[:, 64 * h:64 * h + 64], start=False, stop=False)
                        nc.tensor.matmul(pt3[64:128, 64 * h:64 * h + 64],
                                         wt1[:, 128 * h + 64:128 * h + 128],
                                         tmj(vtt, g, h), start=False, stop=True)
                    nc.scalar.activation(ys_t[tt][hf:hf + 64, :], pt3[64:128, :], AF.Copy)
                    if g >= 9 and g % 2 == 1:
                        kb = tt - 4
                        ccs = cc_in[kb][:] if split_cc else cc_in[kb]
                        nc.sync.dma_start(out=ccs, in_=ys_t[tt][:])
                        if split_cc:
                            nc.gpsimd.collective_compute(
                                "AllReduce", mybir.AluOpType.add,
                                replica_groups=[[0, 4], [1, 5], [2, 6], [3, 7]],
                                ins=[cc_in[kb][:].opt()],
                                outs=[cc_out[kb][:].opt()])
                    pst = q_("q2", [128, 256])
                    for h in range(8):
                        i, r0 = h // 2, 64 * (h % 2)
                        nc.tensor.matmul(pst[r0:r0 + 64, 64 * i:64 * i + 64],
                                         tmj(btt, g, h), us[:, 64 * h:64 * h + 64],
                                         start=True, stop=False)
                        nc.tensor.matmul(pst[r0:r0 + 64, 64 * i:64 * i + 64],
                                         tmj(ktt, g, h), tmj(vtt, g, h),
                                         start=False, stop=True)
                    stmp = p3([128, 256], f32, "stmp")
                    nc.vector.tensor_tensor(out=stmp[:], in0=pst[:], in1=s0t[:], op=AL.add)
                    elg = eLa[:, g:g + 1]
                    elb = bass.AP(tensor=elg.tensor, offset=elg.offset,
                                  ap=[list(elg.ap[0]), [G, NC], [0, 64]])
                    nc.vector.tensor_tensor(out=s0t[:].rearrange("p (i d) -> p i d", d=64),
                                            in0=stmp[:].rearrange("p (i d) -> p i d", d=64),
                                            in1=elb, op=AL.mult)
                    if g + 1 < G:
                        wts[g + 1] = masks(g + 1, pw.pop(g + 1))
                    if g + 2 < G:
                        pw[g + 2] = prework(g + 2)
                if not split_cc:
                    nc.gpsimd.collective_compute(
                        "AllReduce", mybir.AluOpType.add,
                        replica_groups=[[0, 4], [1, 5], [2, 6], [3, 7]],
                        ins=[cc_inF[:].opt()], outs=[cc_outF[:].opt()])

            # =================== post ================================
            with tc.tile_pool(name="P4", bufs=1) as P4:
                def p4(shape, dt, tag):
                    return P4.tile(list(shape), dt, tag=tag, name=tag)

                dbg = os.environ.get('BIRWKV_DBG', '')
                for mt in (3, 2, 1, 0):
                    sx = mt % 2
                    cs = p4([128, C], bf16, f"cs{sx}")
                    nc.sync.dma_start(out=cs[:],
                                      in_=cc_out[3 - mt][:] if split_cc else cc_out[3 - mt])
                    dpe = p4([128, C], bf16, f"dpe{sx}")
                    nc.vector.tensor_tensor(out=dpe[:], in0=cs[:], in1=ys_t[7 - mt][:],
                                            op=AL.subtract)
                    prv = q_("q0" if sx == 0 else "q1", [128, C])
                    nc.tensor.matmul(prv[:], rj, dpe[:], start=True, stop=True)
                    rec = p4([128, C], f32, f"rec{sx}")
                    nc.scalar.activation(rec[:], prv[:], AF.Copy)

                    wn = p4([128, C], f32, f"wn{sx}")
                    gb = bass.AP(tensor=gate_t[:].tensor,
                                 offset=gate_t[:, 8 * mt:8 * mt + 8].offset,
                                 ap=[list(gate_t[:].ap[0]), [1, 8], [0, N]])
                    nc.vector.tensor_scalar(out=wn[:].rearrange("p (h d) -> p h d", d=N),
                                            in0=gb, scalar1=dc[:, 1:2], scalar2=dc[:, 0:1],
                                            op0=AL.mult, op1=AL.add)
                    d = p4([128, C], f32, f"cd{sx}")
                    nc.vector.tensor_tensor(out=d[:], in0=ys_t[mt][:], in1=rec[:],
                                            op=AL.subtract)
                    m = p4([128, C], f32, f"cm{sx}")
                    nc.gpsimd.tensor_tensor(out=m[:], in0=d[:], in1=wn[:], op=AL.mult)
                    xo = p4([128, C], f32, f"xo{sx}")
                    nc.vector.tensor_tensor(out=xo[:], in0=rec[:], in1=m[:], op=AL.add)
                    xsq = p4([128, C], f32, f"xsq{sx}")
                    nc.scalar.activation(xsq[:], xo[:], AF.Square)
                    sm = p4([128, 8], f32, f"sm{sx}")
                    s2 = p4([128, 8], f32, f"s2{sx}")
                    nc.vector.tensor_reduce(out=sm[:], in_=xo[:].rearrange("p (h d) -> p h d", d=N),
                                            axis=mybir.AxisListType.X, op=AL.add)
                    nc.vector.tensor_reduce(out=s2[:], in_=xsq[:].rearrange("p (h d) -> p h d", d=N),
                                            axis=mybir.AxisListType.X, op=AL.add)
                    mu = p4([128, 8], f32, f"mu{sx}")
                    nc.vector.tensor_scalar(out=mu[:], in0=sm[:], scalar1=1.0 / N, scalar2=None,
                                            op0=AL.mult)
                    m2 = p4([128, 8], f32, f"m2{sx}")
                    nc.scalar.activation(m2[:], sm[:], AF.Square, scale=1.0 / N)
                    vr = p4([128, 8], f32, f"vr{sx}")
                    nc.vector.scalar_tensor_tensor(out=vr[:], in0=s2[:], scalar=1.0 / N,
                                                   in1=m2[:], op0=AL.mult, op1=AL.subtract)
                    sd = p4([128, 8], f32, f"sd{sx}")
                    nc.scalar.activation(sd[:], vr[:], AF.Sqrt, bias=egn[:])
                    nc.vector.reciprocal(out=sd[:], in_=sd[:])
                    mb = bass.AP(tensor=mu[:].tensor, offset=mu[:].offset,
                                 ap=[list(mu[:].ap[0]), [1, 8], [0, N]])
                    sb = bass.AP(tensor=sd[:].tensor, offset=sd[:].offset,
                                 ap=[list(sd[:].ap[0]), [1, 8], [0, N]])
                    xc = p4([128, C], f32, f"xc{sx}")
                    nc.vector.tensor_tensor(out=xc[:].rearrange("p (h d) -> p h d", d=N),
                                            in0=xo[:].rearrange("p (h d) -> p h d", d=N),
                                            in1=mb, op=AL.subtract)
                    nc.vector.tensor_tensor(out=xc[:].rearrange("p (h d) -> p h d", d=N),
                                            in0=xc[:].rearrange("p (h d) -> p h d", d=N),
                                            in1=sb, op=AL.mult)
                    nc.gpsimd.tensor_tensor(out=xc[:], in0=xc[:], in1=lngb[:], op=AL.mult)
                    nc.vector.tensor_tensor(out=xc[:], in0=xc[:], in1=lnbb[:], op=AL.add)
                    nc.vector.tensor_tensor(out=xc[:], in0=xc[:], in1=resid_t[mt][:], op=AL.add)
                    pgg = q_("q2" if sx == 0 else "q3", [128, C])
                    nc.tensor.matmul(pgg[:], hg_[:, 128 * mt:128 * (mt + 1)], g2t,
                                     start=True, stop=True)
                    xog = p4([128, C], f32, f"xog{sx}")
                    nc.vector.tensor_tensor(out=xog[:], in0=xc[:], in1=pgg[:], op=AL.mult)
                    ptr = q_("q0" if sx == 0 else "q1", [128, C])
                    for i in range(NC):
                        nc.tensor.matmul(ptr[:, 128 * i:128 * (i + 1)],
                                         xog[:, 128 * i:128 * (i + 1)],
                                         idt, is_transpose=True, start=True, stop=True)
                    xogc = p4([128, C], bf16, f"xogc{sx}")
                    nc.scalar.activation(xogc[:], ptr[:], AF.Copy)
                    po = q_("q2" if sx == 0 else "q3", [128, C])
                    for i in range(NC):
                        nc.tensor.matmul(po[:], xogc[:, 128 * i:128 * (i + 1)],
                                         WO(i), start=(i == 0), stop=(i == NC - 1))
                    oo = p4([128, C], f32, f"oo{sx}")
                    nc.scalar.activation(oo[:], po[:], AF.Copy)
                    if dbg == 'ys':
                        nc.scalar.activation(oo[:], ys_t[mt][:], AF.Copy)
                    elif dbg == 'resid':
                        nc.scalar.activation(oo[:], resid_t[mt][:], AF.Copy)
                    elif dbg == 'recv':
                        nc.scalar.activation(oo[:], rec[:], AF.Copy)
                    nc.sync.dma_start(out=out_d[128 * mt:128 * (mt + 1), :], in_=oo[:])
    nc.finalize()
    return nc


WNAMES = ['x_r', 'x_w', 'x_k', 'x_v', 'x_a', 'x_g', 'w0', 'w1', 'w2', 'a0', 'a1',
          'a2', 'v0', 'v1', 'v2', 'g1', 'g2', 'k_k', 'k_a', 'r_k', 'gate_w',
          'ln_g', 'ln_b', 'Wr', 'Wk', 'Wv', 'Wo']


def _host_inputs(x, v_first, weights):
    import ml_dtypes
    bf = ml_dtypes.bfloat16
    (x_r, x_w, x_k, x_v, x_a, x_g, w0, w1, w2, a0, a1, a2, v0, v1, v2, g1, g2,
     k_k, k_a, r_k, gate_w, ln_g, ln_b, Wr, Wk, Wv, Wo) = weights
    f = np.float32
    vecs = np.zeros((C, 12), f)
    for j, v in enumerate([x_r, x_k, x_v, k_k, k_a, 1.0 - k_a.reshape(C), w0, a0, v0,
                           r_k.reshape(C)]):
        vecs[:, j] = np.asarray(v).reshape(-1)[:C]
    gwT = np.zeros((C, 8), f)
    for h in range(H):
        gwT[h * N:(h + 1) * N, h] = gate_w[h]
    ident = np.eye(128, dtype=f)
    revJ = ident[::-1].copy()
    ones2 = np.zeros((128, 2), f); ones2[:64, 0] = 1; ones2[64:, 1] = 1
    exp2 = ones2.T.copy()
    stack = lambda w, mixv: np.concatenate([w, w * np.asarray(mixv).reshape(C, 1)], 0).astype(f)

    def as128(a, rows):
        out = np.zeros((128, a.shape[1]), f)
        out[:rows] = a
        return out

    def fold(a, rows=128):  # (k*rows, c) -> (rows, k*c) blocks
        k = a.shape[0] // rows
        return a.reshape(k, rows, a.shape[1]).transpose(1, 0, 2).reshape(rows, -1)

    # masks for the in-chunk causal selects
    pp = np.arange(64)[:, None]
    dd = np.arange(64)[None, :]
    mw = np.zeros((64, 128), f)
    mw[:, 0:64] = (0 + dd >= pp + 1)   # u=0: strict upper
    mw[:, 64:128] = (1 + dd >= pp + 1)  # u=1: upper incl diag
    mm = (dd <= pp - 1).astype(f)       # strict lower

    blocks = {
        'wr': fold(Wr.T.astype(f)), 'wk': fold(Wk.T.astype(f)),
        'wv': fold(Wv.T.astype(f)), 'wo': fold(Wo.T.astype(f)),
        'w1t': fold(stack(w1, x_w)), 'a1t': fold(stack(a1, x_a)),
        'g1t': fold(stack(g1, x_g)), 'v1t': fold(np.asarray(v1, f)),
        'w2': as128(np.asarray(w2, f), 64), 'a2': as128(np.asarray(a2, f), 64),
        'v2': as128(np.asarray(v2, f), 32), 'g2': np.asarray(g2, f),
        'idb': ident, 'rj': revJ, 'o2': ones2, 'e2': as128(exp2, 2),
        'gw': fold(gwT), 'mw': as128(mw, 64), 'mm': as128(mm, 64),
    }
    wpackb = np.zeros((128, NB), f)
    for nm, rows, w in _PACKB:
        assert blocks[nm].shape == (128, w), (nm, blocks[nm].shape, w)
        wpackb[:, _OFFB[nm]:_OFFB[nm] + w] = blocks[nm]
    wpackb = np.ascontiguousarray(wpackb.astype(bf))

    shared = dict(
        wpbd=wpackb,
        lng=np.asarray(ln_g, f).reshape(1, C), lnb=np.asarray(ln_b, f).reshape(1, C),
    )
    in_maps = []
    for core in range(8):
        d, b = core // 4, core % 4
        xb = x[b]
        xxb = np.vstack([np.zeros((1, C), f), xb[:-1]]) - xb
        vfb = v_first[b]
        if d == 1:
            xb, xxb, vfb = xb[::-1], xxb[::-1], vfb[::-1]
        wpackf = np.zeros((128, NF), f)
        wpackf[:, 0:48] = fold(vecs)
        wpackf[:, 48:176] = ident
        wpackf[:, 176] = float(d)
        wpackf[:, 177] = 1.0 - 2.0 * d
        m = dict(shared)
        m.update(xT=np.ascontiguousarray(xb.T.astype(bf)),
                 xxT=np.ascontiguousarray(xxb.T.astype(bf)),
                 vfT=np.ascontiguousarray(vfb.T.astype(bf)),
                 wpfd=np.ascontiguousarray(wpackf))
        in_maps.append(m)
    return in_maps


def _forward_np(inputs):
    f = np.float32
    x = np.asarray(inputs['x'], f); v_first = np.asarray(inputs['v_first'], f)
    g1 = np.asarray(inputs['g1'], f); g2 = np.asarray(inputs['g2'], f)
    get = lambda n: np.asarray(inputs[n], f)
    x_r, x_w, x_k, x_v, x_a, x_g = [get(n).reshape(C) for n in
                                    ['x_r', 'x_w', 'x_k', 'x_v', 'x_a', 'x_g']]
    w0, a0, v0 = [get(n).reshape(C) for n in ['w0', 'a0', 'v0']]
    w1, w2, a1, a2, v1, v2 = [get(n) for n in ['w1', 'w2', 'a1', 'a2', 'v1', 'v2']]
    k_k, k_a = get('k_k').reshape(C), get('k_a').reshape(C)
    r_k, gate_w = get('r_k'), get('gate_w')
    ln_g, ln_b = get('ln_g'), get('ln_b')
    Wr, Wk, Wv, Wo = get('Wr'), get('Wk'), get('Wv'), get('Wo')
    sig = lambda z: 1.0 / (1.0 + np.exp(-z))
    xx = np.concatenate([np.zeros((B, 1, C), f), x[:, :-1]], axis=1) - x
    xr = x + xx * x_r; xw = x + xx * x_w; xk = x + xx * x_k
    xv = x + xx * x_v; xa = x + xx * x_a; xg = x + xx * x_g
    r = xr @ Wr.T
    logdec = -LAM * sig(w0 + np.tanh(xw @ w1) @ w2)
    k = xk @ Wk.T
    v = xv @ Wv.T
    v = v + (v_first - v) * sig(v0 + (x @ v1) @ v2)
    aicl = sig(a0 + (xa @ a1) @ a2)
    gg = sig(xg @ g1) @ g2
    kk = k * k_k
    kkh = kk.reshape(B, T, H, N)
    nrm = np.maximum(np.linalg.norm(kkh, axis=-1, keepdims=True), 1e-12)
    kkn = (kkh / nrm).reshape(B, T, C)
    ks = k * (1.0 + (aicl - 1.0) * k_a)
    an = -kkn; bn = kkn * aicl

    def mkstream(t):
        th = t.reshape(B, T, H, N)
        return np.concatenate([th, th[:, ::-1]], axis=0).transpose(0, 2, 1, 3) \
                 .reshape(2 * B * H, T, N)

    Z = 2 * B * H
    rs, ws, kss, vs, as_, bs2 = [np.ascontiguousarray(mkstream(t))
                                 for t in (r, logdec, ks, v, an, bn)]
    S = np.zeros((Z, N, N), f)
    ys = np.empty((Z, T, N), f)
    tril_sT = np.tril(np.ones((L, L), f), -1).T
    triu_i = np.triu(np.ones((L, L), f), 0)
    one0 = np.ones((Z, 1, N), f)
    for g in range(G):
        sl = slice(g * L, (g + 1) * L)
        P = np.cumsum(ws[:, sl], axis=1)
        expP = np.exp(P)
        expN = 1.0 / expP
        expPs = np.concatenate([one0, expP[:, :-1]], axis=1)
        rt = rs[:, sl] * expP; at = as_[:, sl] * expPs
        kt = kss[:, sl] * expN; bt = bs2[:, sl] * expN
        vg = vs[:, sl]
        ST = S.transpose(0, 2, 1)
        atT = at.transpose(0, 2, 1); rtT = rt.transpose(0, 2, 1)
        KA = kt @ atT; BA = bt @ atT
        KR = kt @ rtT; BR = bt @ rtT
        GT = KA * tril_sT; MT = BA * tril_sT
        X = at @ ST + GT.transpose(0, 2, 1) @ vg
        Mp = np.ascontiguousarray(MT.transpose(0, 2, 1))
        for lev in range(6):
            X += Mp @ X
            if lev < 5:
                Mp = Mp @ Mp
        ys[:, sl] = (rt @ ST + (BR * triu_i).transpose(0, 2, 1) @ X
                     + (KR * triu_i).transpose(0, 2, 1) @ vg)
        S = ((ST + bt.transpose(0, 2, 1) @ X + kt.transpose(0, 2, 1) @ vg)
             * expP[:, -1][:, :, None]).transpose(0, 2, 1)
    ysh = ys.reshape(2 * B, H, T, N).transpose(0, 2, 1, 3)
    ys_f = ysh[:B].reshape(B, T, C)
    ys_b = np.ascontiguousarray(ysh[B:, ::-1]).reshape(B, T, C)
    gate = sig(np.einsum('bthn,hn->bth', xx.reshape(B, T, H, N), gate_w))
    gate = np.repeat(gate, N, axis=2)
    xo = gate * ys_f + (1.0 - gate) * ys_b
    xoh = xo.reshape(B, T, H, N)
    mu = xoh.mean(-1, keepdims=True); var = xoh.var(-1, keepdims=True)
    xoh = (xoh - mu) / np.sqrt(var + EPS_GN)
    xoh = xoh * ln_g.reshape(H, N) + ln_b.reshape(H, N)
    resid = ((r.reshape(B, T, H, N) * ks.reshape(B, T, H, N) * r_k)
             .sum(-1, keepdims=True) * v.reshape(B, T, H, N))
    xo = xoh.reshape(B, T, C) + resid.reshape(B, T, C)
    return ((xo * gg).reshape(B * T, C) @ Wo.T).reshape(B, T, C)


def _run_bass(inputs, trace=False):
    from concourse.bass_utils import run_bass_kernel_spmd
    x = np.asarray(inputs['x'], np.float32)
    v_first = np.asarray(inputs['v_first'], np.float32)
    weights = [np.asarray(inputs[n], np.float32) for n in WNAMES]
    in_maps = _host_inputs(x, v_first, weights)
    if 'nc' not in _cache:
        _cache['nc'] = _build()
    res = run_bass_kernel_spmd(_cache['nc'], in_maps, list(range(8)), trace=trace)
    outs = res.results
    out = np.zeros((B, T, C), np.float32)
    for core in range(8):
        d, b = core // 4, core % 4
        part = np.asarray(outs[core]['out'])
        if d == 0:
            out[b, 0:TH] = part
        else:
            out[b, TH:T] = part[::-1]
    _cache['last_res'] = res
    return out


def kernel(trace=False, **inputs):
    v_first = np.asarray(inputs['v_first'], np.float32)
    if not os.environ.get('BIRWKV_NO_BASS'):
        try:
            out = _run_bass(inputs, trace=trace)
            return np.stack([out, v_first]).astype(np.float32)
        except Exception:
            import traceback; traceback.print_exc()
    out = _forward_np(inputs)
    return np.stack([out, v_first]).astype(np.float32)
